# revision 2
# baseline (speedup 1.0000x reference)
"""AttnPooling Trainium2 kernel: 8-core data-parallel over B*N items.

Per item (b,n): x is (D=128, K=4096) fp32, K contiguous in DRAM.
  mean   = sum_k x[:,k]*m[k] / c           (c = sum m)
  query  = Wq @ mean + bq
  v      = Wk^T query = (Wk^T Wq) mean + Wk^T bq
  s_k    = v . x_k   (+ const that cancels in softmax; bk unused)
  p_k    = m_k exp(s_k/sqrt(D));  out = sum_k p_k x_k / sum_k p_k

Trick: Xm = x * (m/c) broadcast is built once per item as the byproduct of the
mean's fused tensor_tensor_reduce; the scores matmuls then use Xm, so
exp(alpha * s'_k) with alpha=c/sqrt(D) gives exp(s_k/sqrt(D)) at valid k and
exp(0)=1 at masked k; the pooled reduction uses Xm again (masking for free) and
Z is corrected by Zfake - (K - c).
"""

import sys

sys.path.insert(0, "/opt/trn_rl_repo")

import numpy as np
from contextlib import ExitStack

NI = 16  # items per core
D = 128
K = 4096
NCORES = 8
SD = 1.0 / np.sqrt(128.0)  # 1/sqrt(D)

_CACHE = {}


def _build():
    import concourse.bass as bass
    import concourse.tile as tile
    from concourse import bacc, mybir
    from concourse.dve_ops import TENSOR_TENSOR_REDUCE

    dt = mybir.dt
    Alu = mybir.AluOpType
    Act = mybir.ActivationFunctionType

    nc = bacc.Bacc(
        "TRN2", target_bir_lowering=False, debug=False, num_devices=NCORES
    )
    x_d = nc.dram_tensor("x", [NI, D, K], dt.float32, kind="ExternalInput").ap()
    mask_d = nc.dram_tensor("mask", [NI, K], dt.int32, kind="ExternalInput").ap()
    wq_d = nc.dram_tensor("Wq", [D, D], dt.float32, kind="ExternalInput").ap()
    wk_d = nc.dram_tensor("Wk", [D, D], dt.float32, kind="ExternalInput").ap()
    bq_d = nc.dram_tensor("bq", [D, 1], dt.float32, kind="ExternalInput").ap()
    out_d = nc.dram_tensor("out", [D, NI], dt.float32, kind="ExternalOutput").ap()

    CH = 1024  # TTR / broadcast chunk (2 PSUM banks)
    NCH = K // CH

    with tile.TileContext(nc) as tc, ExitStack() as ctx:
        # SBUF pools
        xp = ctx.enter_context(tc.tile_pool(name="xp", bufs=3))
        xmp = ctx.enter_context(tc.tile_pool(name="xmp", bufs=3))
        junk = ctx.enter_context(tc.tile_pool(name="junk", bufs=2))
        mbp = ctx.enter_context(tc.tile_pool(name="mbp", bufs=2))
        accp = ctx.enter_context(tc.tile_pool(name="accp", bufs=6))
        per = ctx.enter_context(tc.tile_pool(name="per", bufs=1))
        # PSUM pools: bc 2banks*2 + sp 1bank*2 + tp 1bank*2 = 8 banks
        bc = ctx.enter_context(tc.tile_pool(name="bc", bufs=2, space="PSUM"))
        sp = ctx.enter_context(tc.tile_pool(name="sp", bufs=2, space="PSUM"))
        tp = ctx.enter_context(tc.tile_pool(name="tp", bufs=2, space="PSUM"))

        # persistent tiles
        wq = per.tile([D, D], dt.float32, tag="wq")
        wk = per.tile([D, D], dt.float32, tag="wk")
        bq = per.tile([D, 1], dt.float32, tag="bq")
        cqk = per.tile([D, D], dt.bfloat16, tag="cqk")
        meanb = per.tile([D, NI], dt.bfloat16, tag="meanb")
        w0 = per.tile([D, 1], dt.float32, tag="w0")
        ones32 = per.tile([32, D], dt.bfloat16, tag="ones32")
        ones32f = per.tile([32, D], dt.float32, tag="ones32f")
        es = [
            per.tile([32, K], dt.bfloat16, tag=f"e{j}", name=f"e{j}")
            for j in range(3)
        ]
        mi32 = per.tile([NI, K], dt.int32, tag="mi32")
        m16 = per.tile([NI, K], dt.bfloat16, tag="m16")
        cinvrow32 = per.tile([32, NI], dt.float32, tag="cinvrow32")
        cinvb = per.tile([D, NI], dt.float32, tag="cinvb")
        c32 = per.tile([32, 32], dt.float32, tag="c32")
        c32t = per.tile([32, 32], dt.float32, tag="c32t")
        cinvrow = per.tile([1, NI], dt.float32, tag="cinvrow")
        zp = per.tile([1, NI * 8], dt.float32, tag="zp")
        mean = per.tile([D, NI], dt.float32, tag="mean")
        vt = per.tile([D, NI], dt.bfloat16, tag="vt")
        praw = per.tile([D, NI], dt.float32, tag="praw")
        zrow = per.tile([1, NI], dt.float32, tag="zrow")
        zinv = per.tile([1, NI], dt.float32, tag="zinv")
        frow32 = per.tile([32, NI], dt.float32, tag="frow32")
        outt = per.tile([D, NI], dt.float32, tag="outt")

        # ---- setup ----
        nc.sync.dma_start(wq[:, :], wq_d[:, :])
        nc.sync.dma_start(wk[:, :], wk_d[:, :])
        nc.sync.dma_start(bq[:, :], bq_d[:, :])
        nc.sync.dma_start(mi32[:, :], mask_d[:, :])
        nc.vector.memset(ones32[:, :], 0.0)
        nc.vector.memset(ones32[0:1, :], 1.0)
        nc.vector.memset(ones32f[:, :], 0.0)
        nc.vector.memset(ones32f[0:1, :], 1.0)
        nc.vector.memset(cinvrow32[:, :], 0.0)
        nc.vector.memset(frow32[:, :], 0.0)
        nc.vector.memset(c32[:, :], 0.0)
        for t in es:
            nc.gpsimd.memset(t[:, :], 0.0)

        # mask -> f32, with per-item valid counts c into c32[:,0]
        nc.scalar.activation(
            m16[:, :], mi32[:, :], Act.Copy, accum_out=c32[0:NI, 0:1]
        )
        # crow (1,NI) = c^T via 32x32 block transpose
        nc.vector.transpose(c32t[:, :], c32[:, :])
        crow = c32t[0:1, 0:NI]
        nc.vector.reciprocal(cinvrow[:, :], crow)
        # cinvb (D,NI) = per-partition replicated 1/c (exact fp32 scaling of v)
        nc.vector.tensor_copy(cinvrow32[0:1, :], cinvrow[:, :])
        cinvb_ps = tp.tile([D, NI], dt.float32, tag="smallps")
        nc.tensor.matmul(
            cinvb_ps[:, :], ones32f[:, :], cinvrow32[:, :], start=True, stop=True
        )
        nc.scalar.copy(cinvb[:, :], cinvb_ps[:, :])

        # CQK = Wq^T Wk ; w0 = Wk^T bq
        cqk_ps = tp.tile([D, D], dt.float32, tag="smallps")
        nc.tensor.matmul(cqk_ps[:, :], wq[:, :], wk[:, :], start=True, stop=True)
        nc.scalar.copy(cqk[:, :], cqk_ps[:, :])
        w0_ps = tp.tile([D, 1], dt.float32, tag="smallps")
        nc.tensor.matmul(w0_ps[:, :], wk[:, :], bq[:, :], start=True, stop=True)
        nc.scalar.copy(w0[:, :], w0_ps[:, :])

        # ---- per-item software pipeline (lag-1: mean(i) || attn(i-1)) ----
        xms = [None] * NI

        def mean_phase(i):
            xt = xp.tile([D, K], dt.float32, tag="x", name=f"x_{i}")
            nc.sync.dma_start(xt[:, :], x_d[i, :, :])
            xm = xmp.tile([D, K], dt.bfloat16, tag="xm", name=f"xm_{i}")
            xms[i] = xm
            mb = mbp.tile([D, K], dt.bfloat16, tag="mb", name=f"mb_{i}")
            nc.scalar.dma_start(mb[0:1, :], m16[i : i + 1, :])
            r = 1
            while r < D:
                nc.scalar.dma_start(mb[r : 2 * r, :], mb[0:r, :])
                r *= 2
            nc.vector._custom_dve(
                TENSOR_TENSOR_REDUCE,
                out=xm[:, :],
                in0=xt[:, :],
                in1=mb[:, :],
                s0=0.0,
                s1=1.0,
                accum_out=mean[:, i : i + 1],
            )
            # v_i = (CQK^T . u_i) * (1/c_i) + w0  (all-bf16 matmul keeps FWL alive)
            nc.vector.tensor_copy(meanb[:, i : i + 1], mean[:, i : i + 1])
            vps = tp.tile([D, 1], dt.float32, tag="smallps", name=f"vps_{i}")
            nc.tensor.matmul(
                vps[:, :], cqk[:, :], meanb[:, i : i + 1], start=True, stop=True
            )
            nc.vector.tensor_scalar(
                vt[:, i : i + 1],
                vps[:, :],
                cinvb[:, i : i + 1],
                w0[:, 0:1],
                op0=Alu.mult,
                op1=Alu.add,
            )

        def attn_phase(i):
            xm = xms[i]
            et = es[i % 3]
            for c in range(8):
                st = sp.tile([1, 512], dt.float32, tag="s", name=f"st_{i}_{c}")
                nc.tensor.matmul(
                    st[:, :],
                    vt[:, i : i + 1],
                    xm[:, c * 512 : (c + 1) * 512],
                    start=True,
                    stop=True,
                )
                nc.scalar.activation(
                    et[0:1, c * 512 : (c + 1) * 512],
                    st[:, :],
                    Act.Exp,
                    scale=SD,
                    accum_out=zp[0:1, 8 * i + c : 8 * i + c + 1],
                )
            acc_prev = None
            for c in range(NCH):
                bt = bc.tile([D, CH], dt.float32, tag="bcast", name=f"eb_{i}_{c}")
                for h in range(CH // 512):
                    lo = c * CH + h * 512
                    nc.tensor.matmul(
                        bt[:, h * 512 : (h + 1) * 512],
                        ones32[:, :],
                        et[:, lo : lo + 512],
                        start=True,
                        stop=True,
                    )
                jt = junk.tile([D, CH], dt.bfloat16, tag="junk", name=f"j_{i}_{c}")
                acc = (
                    praw[:, i : i + 1]
                    if c == NCH - 1
                    else accp.tile([D, 1], dt.float32, tag="pacc", name=f"pa_{i}_{c}")
                )
                nc.vector._custom_dve(
                    TENSOR_TENSOR_REDUCE,
                    out=jt[:, :],
                    in0=xm[:, c * CH : (c + 1) * CH],
                    in1=bt[:, :],
                    s0=(0.0 if acc_prev is None else acc_prev[:, 0:1]),
                    s1=1.0,
                    accum_out=acc[:, 0:1],
                )
                acc_prev = acc

        for i in range(NI):
            mean_phase(i)
            if i >= 1:
                attn_phase(i - 1)
        attn_phase(NI - 1)

        # ---- finalize: out = praw * c/Z with Z = Zfake - (K - c) ----
        zp3 = zp[:, :].rearrange("a (n c) -> a n c", c=8)
        nc.vector.tensor_reduce(
            zrow[:, :], zp3, axis=mybir.AxisListType.X, op=Alu.add
        )
        # zrow = (zfake + c) - K
        nc.vector.tensor_tensor(zrow[:, :], zrow[:, :], crow, op=Alu.add)
        nc.vector.tensor_scalar(
            zrow[:, :], zrow[:, :], -float(K), None, op0=Alu.add
        )
        nc.vector.reciprocal(zinv[:, :], zrow[:, :])
        nc.vector.tensor_copy(frow32[0:1, :], zinv[:, :])
        fb = tp.tile([D, NI], dt.float32, tag="smallps")
        nc.tensor.matmul(
            fb[:, :], ones32f[:, :], frow32[:, :], start=True, stop=True
        )
        nc.vector.tensor_tensor(outt[:, :], praw[:, :], fb[:, :], op=Alu.mult)
        nc.sync.dma_start(out_d[:, :], outt[:, :])

    nc.compile()
    return nc


def _get_nc():
    if "nc" not in _CACHE:
        _CACHE["nc"] = _build()
    return _CACHE["nc"]


def _make_in_maps(inputs):
    x, mask = inputs["x"], inputs["mask"]
    Wq, bq, Wk = inputs["Wq"], inputs["bq"], inputs["Wk"]
    B, N, d, H, W = x.shape
    xr = np.ascontiguousarray(x.reshape(B * N, d, H * W).astype(np.float32))
    mr = np.ascontiguousarray(mask.reshape(B * N, H * W).astype(np.int32))
    bq2 = np.ascontiguousarray(bq.reshape(d, 1).astype(np.float32))
    wqc = np.ascontiguousarray(Wq.astype(np.float32))
    wkc = np.ascontiguousarray(Wk.astype(np.float32))
    in_maps = []
    for c in range(NCORES):
        s = slice(c * NI, (c + 1) * NI)
        in_maps.append(
            {
                "x": np.ascontiguousarray(xr[s]),
                "mask": np.ascontiguousarray(mr[s]),
                "Wq": wqc,
                "Wk": wkc,
                "bq": bq2,
            }
        )
    return in_maps


def _gather(res, inputs):
    B, N, d = inputs["x"].shape[:3]
    parts = [np.asarray(res.results[c]["out"]).T for c in range(NCORES)]
    return np.concatenate(parts, axis=0).reshape(B, N, d).astype(np.float32)


def kernel(x, mask, Wq, bq, Wk, bk):
    from concourse.bass_utils import run_bass_kernel_spmd

    inputs = {"x": x, "mask": mask, "Wq": Wq, "bq": bq, "Wk": Wk, "bk": bk}
    nc = _get_nc()
    in_maps = _make_in_maps(inputs)
    res = run_bass_kernel_spmd(nc, in_maps, core_ids=list(range(NCORES)))
    return _gather(res, inputs)



# revision 8
# speedup vs baseline: 2.2370x; 2.2370x over previous
"""AttnPooling Trainium2 kernel: 8-core data-parallel, transposed-token layout.

Per item (of NI=16 per core): x is (D=128, K=4096) fp32 in HBM, host-packed to
bf16 "Xt" layout: SBUF tile (128 part, 32 blocks x [128 x-cols | 1 ones | 1 pad])
where element [p, t*130+d] = x[d, t*128+p].  Token k = t*128+p lives as a
length-128 d-row segment on partition p.

  mean_raw^T (1,129) = sum_t  mfold[:,t]^T @ XT[:, t-block]      (PE, k-contract)
                       col 128 = c (mask count, from the ones col)
  vT (1,128) = ((mean_col^T @ CQK) * (1/c)) + w0^T               (PE + DVE STT)
  VB (128,128) = ones  (x)  vT                                   (PE broadcast)
  Q = XT (.) VB-bcast  -> 7-level binary tree sum over d         (DVE, fp16)
  s_fold (128,32);  E = exp(s*SD);  P = mfold (.) E              (ACT + DVE)
  pooled^T|Z (1,129) = sum_t P[:,t]^T @ XT[:, t-block]           (PE, k-contract)
  out row = pooled * (1/Z)                                       (ACT copy+scale)

All heavy reductions run on PE (partition contraction) or the DVE at 2x bf16;
no 1x-rate custom-DVE pass and no on-chip mask/e broadcast materialization.
"""

import sys

sys.path.insert(0, "/opt/trn_rl_repo")

import numpy as np
from contextlib import ExitStack

NI = 16  # items per core
D = 128
K = 4096
T = 32  # k-tiles per item
BL = 130  # padded block width: 128 x-cols + ones col + pad col
NCORES = 8
SD = 1.0 / np.sqrt(128.0)

_CACHE = {}


def _build():
    import concourse.bass as bass
    import concourse.tile as tile
    from concourse import bacc, mybir

    dt = mybir.dt
    Alu = mybir.AluOpType
    Act = mybir.ActivationFunctionType

    nc = bacc.Bacc(
        "TRN2", target_bir_lowering=False, debug=False, num_devices=NCORES
    )
    x_d = nc.dram_tensor("x", [NI, D, T * BL], dt.bfloat16, kind="ExternalInput").ap()
    mf_d = nc.dram_tensor("mf", [D, NI * T * 2], dt.bfloat16, kind="ExternalInput").ap()
    wq_d = nc.dram_tensor("Wq", [D, D], dt.float32, kind="ExternalInput").ap()
    wk_d = nc.dram_tensor("Wk", [D, D], dt.float32, kind="ExternalInput").ap()
    bq_d = nc.dram_tensor("bq", [D, 1], dt.float32, kind="ExternalInput").ap()
    out_d = nc.dram_tensor("out", [1, NI * D], dt.float32, kind="ExternalOutput").ap()

    with tile.TileContext(nc) as tc, ExitStack() as ctx:
        # SBUF pools
        xp = ctx.enter_context(tc.tile_pool(name="xp", bufs=4))
        qp = ctx.enter_context(tc.tile_pool(name="qp", bufs=2))
        rp = ctx.enter_context(tc.tile_pool(name="rp", bufs=2))
        vp = ctx.enter_context(tc.tile_pool(name="vp", bufs=3))
        per = ctx.enter_context(tc.tile_pool(name="per", bufs=1))
        # PSUM pools: exactly 8 banks total
        meanp = ctx.enter_context(tc.tile_pool(name="meanp", bufs=2, space="PSUM"))
        poolp = ctx.enter_context(tc.tile_pool(name="poolp", bufs=2, space="PSUM"))
        chainp = ctx.enter_context(tc.tile_pool(name="chainp", bufs=2, space="PSUM"))
        vbp = ctx.enter_context(tc.tile_pool(name="vbp", bufs=2, space="PSUM"))

        # persistent tiles
        wq = per.tile([D, D], dt.float32, tag="wq")
        wk = per.tile([D, D], dt.float32, tag="wk")
        bq = per.tile([D, 1], dt.float32, tag="bq")
        # MF/P use stride-2 columns so every (128,1) LDWEIGHTS slice is 4B-aligned
        MF = per.tile([D, NI * T * 2], dt.bfloat16, tag="MF")
        cqk = per.tile([D, D], dt.bfloat16, tag="cqk")
        w0T = per.tile([1, D], dt.float32, tag="w0T")
        ones1 = per.tile([1, D], dt.bfloat16, tag="ones1")
        onebb = per.tile([1, 1], dt.bfloat16, tag="onebb")
        R7 = per.tile([D, NI * T], dt.float32, tag="R7")
        E = per.tile([D, NI * T], dt.bfloat16, tag="E")
        P = per.tile([D, NI * T * 2], dt.bfloat16, tag="P")
        cinv = per.tile([1, NI], dt.float32, tag="cinv")
        zinv = per.tile([1, NI], dt.float32, tag="zinv")
        outt = per.tile([1, NI * D], dt.float32, tag="outt")

        # ---- setup ----
        nc.sync.dma_start(wq[:, :], wq_d[:, :])
        nc.sync.dma_start(wk[:, :], wk_d[:, :])
        nc.sync.dma_start(bq[:, :], bq_d[:, :])
        nc.sync.dma_start(MF[:, :], mf_d[:, :])
        nc.vector.memset(ones1[:, :], 1.0)
        nc.vector.memset(onebb[:, :], 1.0)

        cqk_ps = vbp.tile([D, D], dt.float32, tag="vb", name="cqk_ps")
        nc.tensor.matmul(cqk_ps[:, :], wq[:, :], wk[:, :], start=True, stop=True)
        nc.scalar.copy(cqk[:, :], cqk_ps[:, :])
        w0_ps = vbp.tile([1, D], dt.float32, tag="vb", name="w0_ps")
        nc.tensor.matmul(w0_ps[:, :], bq[:, :], wk[:, :], start=True, stop=True)
        nc.scalar.copy(w0T[:, :], w0_ps[:, :])

        xts = [None] * NI
        vbs = [None] * NI

        def mean_phase(i):
            xt = xp.tile([D, T * BL], dt.bfloat16, tag="x", name=f"x_{i}")
            nc.sync.dma_start(xt[:, :], x_d[i, :, :])
            xts[i] = xt
            meanps = meanp.tile([1, D + 1], dt.float32, tag="m", name=f"mps_{i}")
            for t in range(T):
                nc.tensor.matmul(
                    meanps[:, :],
                    MF[:, 2 * (i * T + t) : 2 * (i * T + t) + 1],
                    xt[:, t * BL : t * BL + D + 1],
                    start=(t == 0),
                    stop=(t == T - 1),
                )
            # single PSUM reader: scalar copies [mean | c] to SBUF, then DVE
            # derives 1/c from the SBUF copy
            mT129 = vp.tile([1, D + 1], dt.bfloat16, tag="mT", name=f"mT_{i}")
            nc.scalar.copy(mT129[:, :], meanps[:, :])
            nc.vector.reciprocal(cinv[0:1, i : i + 1], mT129[0:1, D : D + 1])
            mcps = chainp.tile([D, 1], dt.float32, tag="ch", name=f"mc_{i}")
            nc.tensor.matmul(
                mcps[:, :], mT129[0:1, 0:D], onebb[:, :], start=True, stop=True
            )
            mcol = vp.tile([D, 1], dt.bfloat16, tag="mc", name=f"mcol_{i}")
            nc.scalar.copy(mcol[:, :], mcps[:, :])
            vTps = chainp.tile([1, D], dt.float32, tag="ch", name=f"vT_{i}")
            nc.tensor.matmul(vTps[:, :], mcol[:, :], cqk[:, :], start=True, stop=True)
            vTsb = vp.tile([1, D], dt.bfloat16, tag="vTs", name=f"vTs_{i}")
            nc.vector.scalar_tensor_tensor(
                vTsb[:, :],
                vTps[:, :],
                cinv[0:1, i : i + 1],
                w0T[:, :],
                op0=Alu.mult,
                op1=Alu.add,
            )
            vbps = vbp.tile([D, D], dt.float32, tag="vb", name=f"vbp_{i}")
            nc.tensor.matmul(vbps[:, :], ones1[:, :], vTsb[:, :], start=True, stop=True)
            vb = vp.tile([D, D], dt.bfloat16, tag="vbs", name=f"vb_{i}")
            nc.scalar.copy(vb[:, :], vbps[:, :])
            vbs[i] = vb

        def attn_phase(i):
            xt, vb = xts[i], vbs[i]
            x3 = xt[:, :].rearrange("p (t e) -> p t e", e=BL)[:, :, 0:D]
            v3 = vb[:, :].unsqueeze(1).broadcast_to((D, T, D))
            q = qp.tile([D, K], dt.float16, tag="q", name=f"q_{i}")
            nc.vector.tensor_tensor(
                q[:, :].rearrange("p (t d) -> p t d", d=D), x3, v3, op=Alu.mult
            )
            cur, w = q, D
            for lv in range(6):
                w //= 2
                # last level fp32 so the final pair-add operands stay 4B-aligned
                rdt = dt.float32 if lv == 5 else dt.float16
                r = rp.tile([D, T * w], rdt, tag=f"r{lv}", name=f"r{lv}_{i}")
                c3 = cur[:, :].rearrange("p (t d) -> p t d", d=2 * w)
                nc.vector.tensor_tensor(
                    r[:, :].rearrange("p (t d) -> p t d", d=w),
                    c3[:, :, 0:w],
                    c3[:, :, w : 2 * w],
                    op=Alu.add,
                )
                cur = r
            c3 = cur[:, :].rearrange("p (t d) -> p t d", d=2)
            nc.vector.tensor_tensor(
                R7[:, i * T : (i + 1) * T].unsqueeze(2),
                c3[:, :, 0:1],
                c3[:, :, 1:2],
                op=Alu.add,
            )
            nc.scalar.activation(
                E[:, i * T : (i + 1) * T],
                R7[:, i * T : (i + 1) * T],
                Act.Exp,
                scale=SD,
            )
            mf3 = MF[:, :].rearrange("p (j k) -> p j k", k=2)
            p3 = P[:, :].rearrange("p (j k) -> p j k", k=2)
            nc.vector.tensor_tensor(
                p3[:, i * T : (i + 1) * T, 0:1],
                E[:, i * T : (i + 1) * T].unsqueeze(2),
                mf3[:, i * T : (i + 1) * T, 0:1],
                op=Alu.mult,
            )
            pps = poolp.tile([1, D + 1], dt.float32, tag="p", name=f"pps_{i}")
            for t in range(T):
                nc.tensor.matmul(
                    pps[:, :],
                    P[:, 2 * (i * T + t) : 2 * (i * T + t) + 1],
                    xt[:, t * BL : t * BL + D + 1],
                    start=(t == 0),
                    stop=(t == T - 1),
                )
            # single PSUM reader: scalar copies [pooled | Z] to SBUF first
            po129 = vp.tile([1, D + 1], dt.float32, tag="po", name=f"po_{i}")
            nc.scalar.copy(po129[:, :], pps[:, :])
            nc.vector.reciprocal(zinv[0:1, i : i + 1], po129[0:1, D : D + 1])
            nc.scalar.activation(
                outt[0:1, i * D : (i + 1) * D],
                po129[0:1, 0:D],
                Act.Copy,
                scale=zinv[0:1, i : i + 1],
            )

        for i in range(NI):
            mean_phase(i)
            if i >= 1:
                attn_phase(i - 1)
        attn_phase(NI - 1)

        nc.sync.dma_start(out_d[:, :], outt[:, :])

    nc.compile()
    return nc


def _get_nc():
    if "nc" not in _CACHE:
        _CACHE["nc"] = _build()
    return _CACHE["nc"]


def _pack_inputs(x, mask):
    """Host-side layout prep: bf16 Xt-fold with ones/pad columns, mask fold."""
    import ml_dtypes

    bf16 = ml_dtypes.bfloat16
    B, N, d, H, W = x.shape
    M = B * N  # 128 items
    xr = np.asarray(x, dtype=np.float32).reshape(M, d, T, d)  # [item, d, t, p]
    xt = np.transpose(xr, (0, 3, 2, 1))  # [item, p, t, d]
    xtp = np.zeros((M, d, T, BL), dtype=bf16)
    xtp[:, :, :, 0:d] = xt.astype(bf16)
    xtp[:, :, :, d] = np.asarray(1.0, dtype=bf16)
    xtp = xtp.reshape(M, d, T * BL)

    mr = np.asarray(mask, dtype=np.float32).reshape(M, T, d)  # [item, t, p]
    mfo = np.transpose(mr, (0, 2, 1)).astype(bf16)  # [item, p, t]
    return xtp, mfo


def _make_in_maps(inputs):
    x, mask = inputs["x"], inputs["mask"]
    Wq, bq, Wk = inputs["Wq"], inputs["bq"], inputs["Wk"]
    xtp, mfo = _pack_inputs(x, mask)
    wqc = np.ascontiguousarray(Wq.astype(np.float32))
    wkc = np.ascontiguousarray(Wk.astype(np.float32))
    bq2 = np.ascontiguousarray(bq.reshape(D, 1).astype(np.float32))
    in_maps = []
    for c in range(NCORES):
        s = slice(c * NI, (c + 1) * NI)
        mfc = np.zeros((D, NI * T, 2), dtype=mfo.dtype)
        mfc[:, :, 0] = np.transpose(mfo[s], (1, 0, 2)).reshape(D, NI * T)
        mfc = np.ascontiguousarray(mfc.reshape(D, NI * T * 2))
        in_maps.append(
            {
                "x": np.ascontiguousarray(xtp[s]),
                "mf": mfc,
                "Wq": wqc,
                "Wk": wkc,
                "bq": bq2,
            }
        )
    return in_maps


def _gather(res, inputs):
    B, N, d = inputs["x"].shape[:3]
    parts = [
        np.asarray(res.results[c]["out"], dtype=np.float32).reshape(NI, d)
        for c in range(NCORES)
    ]
    return np.concatenate(parts, axis=0).reshape(B, N, d)


def kernel(x, mask, Wq, bq, Wk, bk):
    from concourse.bass_utils import run_bass_kernel_spmd

    inputs = {"x": x, "mask": mask, "Wq": Wq, "bq": bq, "Wk": Wk, "bk": bk}
    nc = _get_nc()
    in_maps = _make_in_maps(inputs)
    res = run_bass_kernel_spmd(nc, in_maps, core_ids=list(range(NCORES)))
    return _gather(res, inputs)


# revision 16
# speedup vs baseline: 2.4445x; 1.0927x over previous
"""AttnPooling Trainium2 kernel: 8-core data-parallel, transposed-token layout.

Per item (of NI=16 per core): x is (D=128, K=4096) fp32 in HBM, host-packed to
bf16 "Xt" layout: SBUF tile (128 part, 32 blocks x [128 x-cols | 1 ones | 1 pad])
where element [p, t*130+d] = x[d, t*128+p].  Token k = t*128+p lives as a
length-128 d-row segment on partition p.

  mean_raw^T (1,129) = sum_t  mfold[:,t]^T @ XT[:, t-block]      (PE, k-contract)
                       col 128 = c (mask count, from the ones col)
  vT (1,128) = ((mean_col^T @ CQK) * (1/c)) + w0^T               (PE + DVE STT)
  VB (128,128) = ones  (x)  vT                                   (PE broadcast)
  Q = XT (.) VB-bcast  -> 7-level binary tree sum over d         (DVE, fp16)
  s_fold (128,32);  E = exp(s*SD);  P = mfold (.) E              (ACT + DVE)
  pooled^T|Z (1,129) = sum_t P[:,t]^T @ XT[:, t-block]           (PE, k-contract)
  out row = pooled * (1/Z)                                       (ACT copy+scale)

All heavy reductions run on PE (partition contraction) or the DVE at 2x bf16;
no 1x-rate custom-DVE pass and no on-chip mask/e broadcast materialization.
"""

import sys

sys.path.insert(0, "/opt/trn_rl_repo")

import numpy as np
from contextlib import ExitStack

NI = 16  # items per core
D = 128
K = 4096
T = 32  # k-tiles per item
BL = 130  # padded block width: 128 x-cols + ones col + pad col
QW = 136  # padded product-block width (keeps pool view non-coalescible)
NCORES = 8
SD = 1.0 / np.sqrt(128.0)

_CACHE = {}


def _build():
    import concourse.bass as bass
    import concourse.tile as tile
    from concourse import bacc, mybir

    dt = mybir.dt
    Alu = mybir.AluOpType
    Act = mybir.ActivationFunctionType

    nc = bacc.Bacc(
        "TRN2", target_bir_lowering=False, debug=False, num_devices=NCORES
    )
    x_d = nc.dram_tensor("x", [NI, D, T * BL], dt.bfloat16, kind="ExternalInput").ap()
    mf_d = nc.dram_tensor("mf", [D, NI * T * 2], dt.bfloat16, kind="ExternalInput").ap()
    wq_d = nc.dram_tensor("Wq", [D, D], dt.float32, kind="ExternalInput").ap()
    wk_d = nc.dram_tensor("Wk", [D, D], dt.float32, kind="ExternalInput").ap()
    bq_d = nc.dram_tensor("bq", [D, 1], dt.float32, kind="ExternalInput").ap()
    out_d = nc.dram_tensor("out", [1, NI * D], dt.float32, kind="ExternalOutput").ap()

    with tile.TileContext(nc) as tc, ExitStack() as ctx:
        # SBUF pools
        xp = ctx.enter_context(tc.tile_pool(name="xp", bufs=4))
        qp = ctx.enter_context(tc.tile_pool(name="qp", bufs=2))
        rp = ctx.enter_context(tc.tile_pool(name="rp", bufs=2))
        vp = ctx.enter_context(tc.tile_pool(name="vp", bufs=3))
        per = ctx.enter_context(tc.tile_pool(name="per", bufs=1))
        # PSUM pools: exactly 8 banks total
        meanp = ctx.enter_context(tc.tile_pool(name="meanp", bufs=2, space="PSUM"))
        poolp = ctx.enter_context(tc.tile_pool(name="poolp", bufs=2, space="PSUM"))
        chainp = ctx.enter_context(tc.tile_pool(name="chainp", bufs=2, space="PSUM"))
        vbp = ctx.enter_context(tc.tile_pool(name="vbp", bufs=2, space="PSUM"))

        # persistent tiles
        wq = per.tile([D, D], dt.float32, tag="wq")
        wk = per.tile([D, D], dt.float32, tag="wk")
        bq = per.tile([D, 1], dt.float32, tag="bq")
        # MF/P use stride-2 columns so every (128,1) LDWEIGHTS slice is 4B-aligned
        MF = per.tile([D, NI * T * 2], dt.bfloat16, tag="MF")
        cqk = per.tile([D, D], dt.bfloat16, tag="cqk")
        w0T = per.tile([1, D], dt.float32, tag="w0T")
        ones1 = per.tile([1, D], dt.bfloat16, tag="ones1")
        onebb = per.tile([1, 1], dt.bfloat16, tag="onebb")
        R7 = per.tile([D, NI * T], dt.float32, tag="R7")
        E = per.tile([D, NI * T], dt.bfloat16, tag="E")
        P = per.tile([D, NI * T * 2], dt.bfloat16, tag="P")
        cinv = per.tile([1, NI], dt.float32, tag="cinv")
        zinv = per.tile([1, NI], dt.float32, tag="zinv")
        outt = per.tile([1, NI * D], dt.float32, tag="outt")

        # ---- setup ----
        nc.sync.dma_start(wq[:, :], wq_d[:, :])
        nc.sync.dma_start(wk[:, :], wk_d[:, :])
        nc.sync.dma_start(bq[:, :], bq_d[:, :])
        nc.sync.dma_start(MF[:, :], mf_d[:, :])
        nc.vector.memset(ones1[:, :], 1.0)
        nc.vector.memset(onebb[:, :], 1.0)

        cqk_ps = vbp.tile([D, D], dt.float32, tag="vb", name="cqk_ps")
        nc.tensor.matmul(cqk_ps[:, :], wq[:, :], wk[:, :], start=True, stop=True)
        nc.scalar.copy(cqk[:, :], cqk_ps[:, :])
        w0_ps = vbp.tile([1, D], dt.float32, tag="vb", name="w0_ps")
        nc.tensor.matmul(w0_ps[:, :], bq[:, :], wk[:, :], start=True, stop=True)
        nc.scalar.copy(w0T[:, :], w0_ps[:, :])

        # per-item 1/c at setup: c = sum over (p,t) of the mask fold
        one128f = per.tile([D, 1], dt.float32, tag="one128f")
        cpart = per.tile([D, NI], dt.float32, tag="cpart")
        nc.vector.memset(one128f[:, :], 1.0)
        mf3s = MF[:, :].rearrange("p (i t k) -> p i (t k)", i=NI, k=2)
        nc.vector.tensor_reduce(
            cpart[:, :], mf3s, axis=mybir.AxisListType.X, op=Alu.add
        )
        crow_ps = chainp.tile([1, NI], dt.float32, tag="ch", name="crow_ps")
        nc.tensor.matmul(
            crow_ps[:, :], one128f[:, :], cpart[:, :], start=True, stop=True
        )
        nc.vector.reciprocal(cinv[:, :], crow_ps[:, :])

        xts = [None] * NI
        vbs = [None] * NI

        def load_phase(i):
            xt = xp.tile([D, T * BL], dt.bfloat16, tag="x", name=f"x_{i}")
            nc.sync.dma_start(xt[:, :], x_d[i, :, :])
            xts[i] = xt

        def mean_phase(i):
            xt = xts[i]
            meanps = meanp.tile([1, D], dt.float32, tag="m", name=f"mps_{i}")
            for t in range(T):
                nc.tensor.matmul(
                    meanps[:, :],
                    MF[:, 2 * (i * T + t) : 2 * (i * T + t) + 1],
                    xt[:, t * BL : t * BL + D],
                    start=(t == 0),
                    stop=(t == T - 1),
                )
            meanT = vp.tile([1, D], dt.bfloat16, tag="mT", name=f"mT_{i}")
            nc.scalar.copy(meanT[:, :], meanps[:, :])
            mcps = chainp.tile([D, 1], dt.float32, tag="ch", name=f"mc_{i}")
            nc.tensor.matmul(
                mcps[:, :], meanT[:, :], onebb[:, :], start=True, stop=True
            )
            mcol = vp.tile([D, 1], dt.bfloat16, tag="mc", name=f"mcol_{i}")
            nc.scalar.copy(mcol[:, :], mcps[:, :])
            vTps = chainp.tile([1, D], dt.float32, tag="ch", name=f"vT_{i}")
            nc.tensor.matmul(vTps[:, :], mcol[:, :], cqk[:, :], start=True, stop=True)
            vTsb = vp.tile([1, D], dt.bfloat16, tag="vTs", name=f"vTs_{i}")
            nc.vector.scalar_tensor_tensor(
                vTsb[:, :],
                vTps[:, :],
                cinv[0:1, i : i + 1],
                w0T[:, :],
                op0=Alu.mult,
                op1=Alu.add,
            )
            vbps = vbp.tile([D, D], dt.float32, tag="vb", name=f"vbp_{i}")
            nc.tensor.matmul(vbps[:, :], ones1[:, :], vTsb[:, :], start=True, stop=True)
            vb = vp.tile([D, D], dt.bfloat16, tag="vbs", name=f"vb_{i}")
            nc.scalar.copy(vb[:, :], vbps[:, :])
            vbs[i] = vb

        def attn_phase(i):
            xt, vb = xts[i], vbs[i]
            x3 = xt[:, :].rearrange("p (t e) -> p t e", e=BL)[:, :, 0:D]
            v3 = vb[:, :].unsqueeze(1).broadcast_to((D, T, D))
            q = qp.tile([D, K], dt.bfloat16, tag="q", name=f"q_{i}")
            nc.vector.tensor_tensor(
                q[:, :].rearrange("p (t d) -> p t d", d=D), x3, v3, op=Alu.mult
            )
            cur, w = q, D
            for lv in range(6):
                w //= 2
                # last level fp32 so the final pair-add operands stay 4B-aligned
                rdt = dt.float32 if lv == 5 else dt.float16
                r = rp.tile([D, T * w], rdt, tag=f"r{lv}", name=f"r{lv}_{i}")
                c3 = cur[:, :].rearrange("p (t d) -> p t d", d=2 * w)
                nc.vector.tensor_tensor(
                    r[:, :].rearrange("p (t d) -> p t d", d=w),
                    c3[:, :, 0:w],
                    c3[:, :, w : 2 * w],
                    op=Alu.add,
                )
                cur = r
            c3 = cur[:, :].rearrange("p (t d) -> p t d", d=2)
            nc.vector.tensor_tensor(
                R7[:, i * T : (i + 1) * T].unsqueeze(2),
                c3[:, :, 0:1],
                c3[:, :, 1:2],
                op=Alu.add,
            )
            nc.scalar.activation(
                E[:, i * T : (i + 1) * T],
                R7[:, i * T : (i + 1) * T],
                Act.Exp,
                scale=SD,
            )
            mf3 = MF[:, :].rearrange("p (j k) -> p j k", k=2)
            p3 = P[:, :].rearrange("p (j k) -> p j k", k=2)
            nc.vector.tensor_tensor(
                p3[:, i * T : (i + 1) * T, 0:1],
                E[:, i * T : (i + 1) * T].unsqueeze(2),
                mf3[:, i * T : (i + 1) * T, 0:1],
                op=Alu.mult,
            )
            pps = poolp.tile([1, D + 1], dt.float32, tag="p", name=f"pps_{i}")
            for t in range(T):
                nc.tensor.matmul(
                    pps[:, :],
                    P[:, 2 * (i * T + t) : 2 * (i * T + t) + 1],
                    xt[:, t * BL : t * BL + D + 1],
                    start=(t == 0),
                    stop=(t == T - 1),
                )
            # single PSUM reader: scalar copies [pooled | Z] to SBUF first
            po129 = vp.tile([1, D + 1], dt.float32, tag="po", name=f"po_{i}")
            nc.scalar.copy(po129[:, :], pps[:, :])
            nc.vector.reciprocal(zinv[0:1, i : i + 1], po129[0:1, D : D + 1])
            nc.scalar.activation(
                outt[0:1, i * D : (i + 1) * D],
                po129[0:1, 0:D],
                Act.Copy,
                scale=zinv[0:1, i : i + 1],
            )

        load_phase(0)
        for i in range(NI):
            if i + 1 < NI:
                load_phase(i + 1)
            mean_phase(i)
            if i >= 1:
                attn_phase(i - 1)
        attn_phase(NI - 1)

        nc.sync.dma_start(out_d[:, :], outt[:, :])

    nc.compile()
    return nc


def _get_nc():
    if "nc" not in _CACHE:
        _CACHE["nc"] = _build()
    return _CACHE["nc"]


def _pack_inputs(x, mask):
    """Host-side layout prep: bf16 Xt-fold with ones/pad columns, mask fold."""
    import ml_dtypes

    bf16 = ml_dtypes.bfloat16
    B, N, d, H, W = x.shape
    M = B * N  # 128 items
    xr = np.asarray(x, dtype=np.float32).reshape(M, d, T, d)  # [item, d, t, p]
    xt = np.transpose(xr, (0, 3, 2, 1))  # [item, p, t, d]
    xtp = np.zeros((M, d, T, BL), dtype=bf16)
    xtp[:, :, :, 0:d] = xt.astype(bf16)
    xtp[:, :, :, d] = np.asarray(1.0, dtype=bf16)
    xtp = xtp.reshape(M, d, T * BL)

    mr = np.asarray(mask, dtype=np.float32).reshape(M, T, d)  # [item, t, p]
    mfo = np.transpose(mr, (0, 2, 1)).astype(bf16)  # [item, p, t]
    return xtp, mfo


def _make_in_maps(inputs):
    x, mask = inputs["x"], inputs["mask"]
    Wq, bq, Wk = inputs["Wq"], inputs["bq"], inputs["Wk"]
    xtp, mfo = _pack_inputs(x, mask)
    wqc = np.ascontiguousarray(Wq.astype(np.float32))
    wkc = np.ascontiguousarray(Wk.astype(np.float32))
    bq2 = np.ascontiguousarray(bq.reshape(D, 1).astype(np.float32))
    in_maps = []
    for c in range(NCORES):
        s = slice(c * NI, (c + 1) * NI)
        mfc = np.zeros((D, NI * T, 2), dtype=mfo.dtype)
        mfc[:, :, 0] = np.transpose(mfo[s], (1, 0, 2)).reshape(D, NI * T)
        mfc = np.ascontiguousarray(mfc.reshape(D, NI * T * 2))
        in_maps.append(
            {
                "x": np.ascontiguousarray(xtp[s]),
                "mf": mfc,
                "Wq": wqc,
                "Wk": wkc,
                "bq": bq2,
            }
        )
    return in_maps


def _gather(res, inputs):
    B, N, d = inputs["x"].shape[:3]
    parts = [
        np.asarray(res.results[c]["out"], dtype=np.float32).reshape(NI, d)
        for c in range(NCORES)
    ]
    return np.concatenate(parts, axis=0).reshape(B, N, d)


def kernel(x, mask, Wq, bq, Wk, bk):
    from concourse.bass_utils import run_bass_kernel_spmd

    inputs = {"x": x, "mask": mask, "Wq": Wq, "bq": bq, "Wk": Wk, "bk": bk}
    nc = _get_nc()
    in_maps = _make_in_maps(inputs)
    res = run_bass_kernel_spmd(nc, in_maps, core_ids=list(range(NCORES)))
    return _gather(res, inputs)
